# revision 25
# baseline (speedup 1.0000x reference)
"""Binary dense layer  y = x @ sign(W) + b  on 8 Trainium2 NeuronCores.

Problem (hardcoded): x [4096, 4096] f32, W [4096, 4096] f32, b [4096] f32.

Sharding: 2D grid, 4 batch shards x 2 column shards (one core each).
Per core:  xT shard [K=4096, M=1024] f32 (host-pretransposed),
           W shard  [K=4096, N=2048] f32,
           b shard  broadcast to [128, 2048] f32 (host-side layout prep).
On device: x is cast f32->fp16 (DVE), W is binarized with the ScalarE
Sign activation straight to fp16 (+-1 exact in fp16), then a tiled
fp16 matmul (full-precision f32 PSUM accumulation) + f32 bias add.
The host gathers the 8 [1024, 2048] f32 output shards into y[4096, 4096].
"""

import ml_dtypes
import numpy as np

import concourse.bass as bass
import concourse.mybir as mybir
import concourse.tile as tile
from concourse import bacc, bass_utils
from concourse.bass import ds

# ---- problem constants (fixed by the task; kernel.py must be self-contained)
B_FULL = 4096  # batch rows of x
K_FULL = 4096  # contraction dim (n_in)
N_FULL = 4096  # output cols (n_units)
R, C = 4, 2  # batch shards x column shards -> R*C = 8 cores
N_CORES = 8
P = 128

MM_DT = mybir.dt.float16  # matmul dtype: sign(W) is exact, x rounds to 11 bits


def build_nc(m_loc=B_FULL // R, k=K_FULL, n_loc=N_FULL // C,
             n_tile=512, w_kchunk=4):
    """Build + compile the per-core Bass kernel (SPMD: same NEFF on all cores).

    y[m_loc, n_loc] = x[m_loc, k] @ sign(W[k, n_loc]) + b[n_loc]
    with inputs xT = x.T (fp16), w (bf16), bias pre-broadcast to [P, n_loc].

    Wire formats: xT is fp16 (identical to the on-device cast the kernel
    would do anyway), W is bf16 (sign-preserving cast). On device W is
    binarized to +-1 fp16, alternating between ScalarE (Sign activation)
    and VectorE (fused bitwise (w & 0x8000) | 0x3C00 on uint16 views).

    Loop order is k-outer within each n-tile: all m_tiles psum groups
    accumulate in lockstep over k-chunks, so during the prologue the PE
    computes on each arriving x k-chunk + W k-slice immediately instead
    of waiting for the whole x shard.
    """
    ko_n = k // P
    m_tiles = m_loc // P
    n_tiles = n_loc // n_tile
    w_slices = ko_n // w_kchunk

    nc = bacc.Bacc("TRN2", target_bir_lowering=False, debug=False)

    # wire formats are partition-major (host pre-swizzled) so each DMA row
    # is a long contiguous run -> few, large DMA descriptors
    xT = nc.dram_tensor("xT", [P, ko_n, m_loc], mybir.dt.float16,
                        kind="ExternalInput")
    # W arrives as bf16: the f32->bf16 cast preserves sign(W) exactly, and
    # only sign(W) enters the computation -- this halves W DMA traffic.
    w = nc.dram_tensor("w", [P, n_tiles, ko_n, n_tile], mybir.dt.bfloat16,
                       kind="ExternalInput")
    bb = nc.dram_tensor("bias", [P, n_loc], mybir.dt.float32, kind="ExternalInput")
    y = nc.dram_tensor("y", [m_loc, n_loc], mybir.dt.float32, kind="ExternalOutput")

    xT3 = xT.ap()
    w4 = w.ap()
    # output view: row index (mo*P + p) -> [p, mo, n]
    y3 = y.ap().rearrange("(mo p) n -> p mo n", p=P)

    with tile.TileContext(nc) as tc:
        with (
            tc.tile_pool(name="xt_res", bufs=1) as xt_res_pool,
            tc.tile_pool(name="stage", bufs=6) as stage_pool,
            tc.tile_pool(name="wq", bufs=2) as wq_pool,
            tc.tile_pool(name="bias_sb", bufs=1) as bias_pool,
            tc.tile_pool(name="yout", bufs=4) as out_pool,
            tc.tile_pool(name="psum", bufs=8, space="PSUM") as psum_pool,
        ):
            # resident x shard, K on partitions: [P, ko, m], filled by
            # direct DMA in w_kchunk-aligned k-chunks (2KB descriptors)
            xt16 = xt_res_pool.tile([P, ko_n, m_loc], MM_DT)

            def load_x_krange(ko_lo, kos):
                # x loads go via the ACT-sequencer HWDGE ring (ScalarE is
                # otherwise unused) so they drain in parallel with the W
                # loads on the SP ring instead of FIFO-serially behind them
                nc.scalar.dma_start(
                    xt16[:, ds(ko_lo, kos), :],
                    xT3[:, ds(ko_lo, kos), :])

            def load_w_krange(wq, nt, ko_lo, kos, idx):
                # ko-range [P, kos, n_tile] of the nt-th W column tile
                wstage = stage_pool.tile([P, w_kchunk, n_tile], mybir.dt.bfloat16,
                                         tag="wstage", name=f"ws{nt}_{ko_lo}")
                wst = wstage[:, :kos, :]
                nc.sync.dma_start(wst, w4[:, nt, ds(ko_lo, kos), :])
                # DVE fused bitwise binarize: (w & 0x8000) | 0x3C00 on
                # uint16 views == +-1.0 fp16. ~3x faster than ACT Sign,
                # and avoiding the Sign activation drops the const-AP
                # TENSOR_LOAD + ACT_TABLE_LOAD from the kernel preamble.
                dst = wq[:, ds(ko_lo, kos), :]
                nc.vector.tensor_scalar(
                    dst.bitcast(mybir.dt.uint16),
                    wst.bitcast(mybir.dt.uint16),
                    0x8000, 0x3C00,
                    mybir.AluOpType.bitwise_and,
                    mybir.AluOpType.bitwise_or)

            # PE warmup: scratch matmuls keep the PE busy through the DMA
            # prologue so the HAM clock gate is at 8/8 when real matmuls
            # start (otherwise the first ~3.4us of matmuls run at 1.2 GHz)
            n_warm = 12 if ko_n >= 16 else 2
            scratch = xt_res_pool.tile([P, n_tile], MM_DT, name="warm_scratch")
            nc.vector.memset(scratch[:], 0.0)
            ps_warm = psum_pool.tile([P, n_tile], mybir.dt.float32, tag="ps",
                                     name="ps_warm")
            for i in range(n_warm):
                nc.tensor.matmul(ps_warm[:], scratch[:, :P], scratch[:],
                                 start=(i == 0), stop=(i == n_warm - 1))

            # prologue: interleave x k-ranges with W tile 0 k-slices in
            # exactly the order the k-outer loop consumes them; the first
            # slice is split in half so the first real matmul starts sooner.
            # W goes first in each pair: its consumer chain (DMA ->
            # binarize -> MM) is longer than x's (DMA -> MM), and HWDGE
            # DMAs drain in FIFO order.
            wq_tiles = {0: wq_pool.tile([P, ko_n, n_tile], MM_DT, tag="wq",
                                        name="wq0")}
            half = w_kchunk // 2
            ranges = [(0, half), (half, w_kchunk - half)] + [
                (kc * w_kchunk, w_kchunk) for kc in range(1, w_slices)]
            for idx, (ko_lo, kos) in enumerate(ranges):
                load_w_krange(wq_tiles[0], 0, ko_lo, kos, idx)
                load_x_krange(ko_lo, kos)
            bias_sb = bias_pool.tile([P, n_loc], mybir.dt.float32)
            nc.sync.dma_start(bias_sb[:], bb.ap())

            for nt in range(n_tiles):
                wq = wq_tiles.pop(nt)
                ps_tiles = [
                    psum_pool.tile([P, n_tile], mybir.dt.float32, tag="ps",
                                   name=f"ps{nt}_{mt}")
                    for mt in range(m_tiles)
                ]
                for kc in range(w_slices):
                    # prefetch next W tile one k-slice per k-chunk
                    if nt + 1 < n_tiles:
                        if kc == 0:
                            wq_tiles[nt + 1] = wq_pool.tile(
                                [P, ko_n, n_tile], MM_DT, tag="wq",
                                name=f"wq{nt + 1}")
                        load_w_krange(wq_tiles[nt + 1], nt + 1,
                                      kc * w_kchunk, w_kchunk, kc)
                    last_kc = kc == w_slices - 1
                    for mt in range(m_tiles):
                        for ko in range(kc * w_kchunk, (kc + 1) * w_kchunk):
                            nc.tensor.matmul(
                                ps_tiles[mt][:],
                                xt16[:, ko, ds(mt * P, P)],
                                wq[:, ko, :],
                                start=(ko == 0),
                                stop=(ko == ko_n - 1),
                            )
                        if last_kc:
                            yt = out_pool.tile([P, n_tile], mybir.dt.float32,
                                               tag="yt")
                            nc.vector.tensor_add(
                                yt[:], ps_tiles[mt][:],
                                bias_sb[:, ds(nt * n_tile, n_tile)])
                            # y stores on the ACT ring, decoupled from the
                            # W-prefetch FIFO on the SP ring
                            nc.scalar.dma_start(
                                y3[:, mt, ds(nt * n_tile, n_tile)], yt[:])

    nc.compile()
    return nc


_NC_CACHE = {}


def _get_nc():
    if "nc" not in _NC_CACHE:
        _NC_CACHE["nc"] = build_nc()
    return _NC_CACHE["nc"]


M_LOC = B_FULL // R
N_LOC = N_FULL // C
N_TILE = 512


def wire_x(x_shard, k=K_FULL):
    """[m, k] f32 -> partition-major [P, ko, m] fp16."""
    m = x_shard.shape[0]
    return np.ascontiguousarray(
        x_shard.reshape(m, k // P, P).transpose(2, 1, 0)).astype(np.float16)


def wire_w(w_shard, k=K_FULL, n_tile=N_TILE):
    """[k, n] f32 -> partition-major [P, nt, ko, n_tile] bf16."""
    n = w_shard.shape[1]
    return np.ascontiguousarray(
        w_shard.reshape(k // P, P, n // n_tile, n_tile).transpose(1, 2, 0, 3)
    ).astype(ml_dtypes.bfloat16)


def wire_b(b_shard):
    """[n] f32 -> broadcast [P, n] f32."""
    return np.ascontiguousarray(
        np.broadcast_to(b_shard, (P, b_shard.shape[0])).astype(np.float32))


def make_in_maps(x, W, b):
    """Host-side shard + layout prep: per-core input dicts."""
    x = np.ascontiguousarray(np.asarray(x, dtype=np.float32))
    W = np.ascontiguousarray(np.asarray(W, dtype=np.float32))
    b = np.ascontiguousarray(np.asarray(b, dtype=np.float32))
    in_maps = []
    for core in range(N_CORES):
        i, j = divmod(core, C)
        in_maps.append({
            "xT": wire_x(x[i * M_LOC:(i + 1) * M_LOC, :]),
            "w": wire_w(W[:, j * N_LOC:(j + 1) * N_LOC]),
            "bias": wire_b(b[j * N_LOC:(j + 1) * N_LOC]),
        })
    return in_maps


def gather_out(results):
    """Assemble per-core y shards into the full [4096, 4096] output."""
    y = np.empty((B_FULL, N_FULL), np.float32)
    for core in range(N_CORES):
        i, j = divmod(core, C)
        y[i * M_LOC:(i + 1) * M_LOC, j * N_LOC:(j + 1) * N_LOC] = (
            results[core]["y"])
    return y


def kernel(x, W, b):
    nc = _get_nc()
    in_maps = make_in_maps(x, W, b)
    res = bass_utils.run_bass_kernel_spmd(nc, in_maps, core_ids=list(range(N_CORES)))
    return gather_out(res.results)


# revision 27
# speedup vs baseline: 1.0282x; 1.0282x over previous
"""Binary dense layer  y = x @ sign(W) + b  on 8 Trainium2 NeuronCores.

Problem (hardcoded): x [4096, 4096] f32, W [4096, 4096] f32, b [4096] f32.

Sharding: 2D grid, 4 batch shards x 2 column shards (one core each).
Per core:  xT shard [K=4096, M=1024] f32 (host-pretransposed),
           W shard  [K=4096, N=2048] f32,
           b shard  broadcast to [128, 2048] f32 (host-side layout prep).
On device: x is cast f32->fp16 (DVE), W is binarized with the ScalarE
Sign activation straight to fp16 (+-1 exact in fp16), then a tiled
fp16 matmul (full-precision f32 PSUM accumulation) + f32 bias add.
The host gathers the 8 [1024, 2048] f32 output shards into y[4096, 4096].
"""

import ml_dtypes
import numpy as np

import concourse.bass as bass
import concourse.mybir as mybir
import concourse.tile as tile
from concourse import bacc, bass_utils
from concourse.bass import ds

# ---- problem constants (fixed by the task; kernel.py must be self-contained)
B_FULL = 4096  # batch rows of x
K_FULL = 4096  # contraction dim (n_in)
N_FULL = 4096  # output cols (n_units)
R, C = 4, 2  # batch shards x column shards -> R*C = 8 cores
N_CORES = 8
P = 128

MM_DT = mybir.dt.float16  # matmul dtype: sign(W) is exact, x rounds to 11 bits


def build_nc(m_loc=B_FULL // R, k=K_FULL, n_loc=N_FULL // C,
             n_tile=512, w_kchunk=4):
    """Build + compile the per-core Bass kernel (SPMD: same NEFF on all cores).

    y[m_loc, n_loc] = x[m_loc, k] @ sign(W[k, n_loc]) + b[n_loc]
    with inputs xT = x.T (fp16), w (bf16), bias pre-broadcast to [P, n_loc].

    Wire formats: xT is fp16 (identical to the on-device cast the kernel
    would do anyway), W is bf16 (sign-preserving cast). On device W is
    binarized to +-1 fp16, alternating between ScalarE (Sign activation)
    and VectorE (fused bitwise (w & 0x8000) | 0x3C00 on uint16 views).

    Loop order is k-outer within each n-tile: all m_tiles psum groups
    accumulate in lockstep over k-chunks, so during the prologue the PE
    computes on each arriving x k-chunk + W k-slice immediately instead
    of waiting for the whole x shard.
    """
    ko_n = k // P
    m_tiles = m_loc // P
    n_tiles = n_loc // n_tile
    w_slices = ko_n // w_kchunk

    nc = bacc.Bacc("TRN2", target_bir_lowering=False, debug=False)

    # wire formats are partition-major (host pre-swizzled) so each DMA row
    # is a long contiguous run -> few, large DMA descriptors
    xT = nc.dram_tensor("xT", [P, ko_n, m_loc], mybir.dt.float16,
                        kind="ExternalInput")
    # W arrives as bf16: the f32->bf16 cast preserves sign(W) exactly, and
    # only sign(W) enters the computation -- this halves W DMA traffic.
    w = nc.dram_tensor("w", [P, n_tiles, ko_n, n_tile], mybir.dt.bfloat16,
                       kind="ExternalInput")
    bb = nc.dram_tensor("bias", [P, n_loc], mybir.dt.float32, kind="ExternalInput")
    y = nc.dram_tensor("y", [m_loc, n_loc], mybir.dt.float32, kind="ExternalOutput")

    xT3 = xT.ap()
    w4 = w.ap()
    # output view: row index (mo*P + p) -> [p, mo, n]
    y3 = y.ap().rearrange("(mo p) n -> p mo n", p=P)

    with tile.TileContext(nc) as tc:
        with (
            tc.tile_pool(name="xt_res", bufs=1) as xt_res_pool,
            tc.tile_pool(name="stage", bufs=6) as stage_pool,
            tc.tile_pool(name="wq", bufs=2) as wq_pool,
            tc.tile_pool(name="bias_sb", bufs=1) as bias_pool,
            tc.tile_pool(name="yout", bufs=4) as out_pool,
            tc.tile_pool(name="psum", bufs=8, space="PSUM") as psum_pool,
        ):
            # resident x shard, K on partitions: [P, ko, m], filled by
            # direct DMA in w_kchunk-aligned k-chunks (2KB descriptors)
            xt16 = xt_res_pool.tile([P, ko_n, m_loc], MM_DT)

            def load_x_krange(ko_lo, kos):
                nc.sync.dma_start(
                    xt16[:, ds(ko_lo, kos), :],
                    xT3[:, ds(ko_lo, kos), :])

            def load_w_krange(wq, nt, ko_lo, kos, idx):
                # ko-range [P, kos, n_tile] of the nt-th W column tile
                wstage = stage_pool.tile([P, w_kchunk, n_tile], mybir.dt.bfloat16,
                                         tag="wstage", name=f"ws{nt}_{ko_lo}")
                wst = wstage[:, :kos, :]
                nc.sync.dma_start(wst, w4[:, nt, ds(ko_lo, kos), :])
                # DVE fused bitwise binarize: (w & 0x8000) | 0x3C00 on
                # uint16 views == +-1.0 fp16. ~3x faster than ACT Sign,
                # and avoiding the Sign activation drops the const-AP
                # TENSOR_LOAD + ACT_TABLE_LOAD from the kernel preamble.
                dst = wq[:, ds(ko_lo, kos), :]
                nc.vector.tensor_scalar(
                    dst.bitcast(mybir.dt.uint16),
                    wst.bitcast(mybir.dt.uint16),
                    0x8000, 0x3C00,
                    mybir.AluOpType.bitwise_and,
                    mybir.AluOpType.bitwise_or)

            # PE warmup: scratch matmuls keep the PE busy through the DMA
            # prologue so the HAM clock gate is at 8/8 when real matmuls
            # start (otherwise the first ~3.4us of matmuls run at 1.2 GHz)
            n_warm = 12 if ko_n >= 16 else 2
            scratch = xt_res_pool.tile([P, n_tile], MM_DT, name="warm_scratch")
            nc.vector.memset(scratch[:], 0.0)
            ps_warm = psum_pool.tile([P, n_tile], mybir.dt.float32, tag="ps",
                                     name="ps_warm")
            for i in range(n_warm):
                nc.tensor.matmul(ps_warm[:], scratch[:, :P], scratch[:],
                                 start=(i == 0), stop=(i == n_warm - 1))

            # prologue: interleave x k-ranges with W tile 0 k-slices in
            # exactly the order the k-outer loop consumes them; the first
            # slice is split in half so the first real matmul starts sooner.
            # W goes first in each pair: its consumer chain (DMA ->
            # binarize -> MM) is longer than x's (DMA -> MM), and HWDGE
            # DMAs drain in FIFO order.
            wq_tiles = {0: wq_pool.tile([P, ko_n, n_tile], MM_DT, tag="wq",
                                        name="wq0")}
            half = w_kchunk // 2
            ranges = [(0, half), (half, w_kchunk - half)] + [
                (kc * w_kchunk, w_kchunk) for kc in range(1, w_slices)]
            for idx, (ko_lo, kos) in enumerate(ranges):
                load_w_krange(wq_tiles[0], 0, ko_lo, kos, idx)
                load_x_krange(ko_lo, kos)
            bias_sb = bias_pool.tile([P, n_loc], mybir.dt.float32)
            nc.sync.dma_start(bias_sb[:], bb.ap())

            for nt in range(n_tiles):
                wq = wq_tiles.pop(nt)
                ps_tiles = [
                    psum_pool.tile([P, n_tile], mybir.dt.float32, tag="ps",
                                   name=f"ps{nt}_{mt}")
                    for mt in range(m_tiles)
                ]
                for kc in range(w_slices):
                    # prefetch next W tile one k-slice per k-chunk
                    if nt + 1 < n_tiles:
                        if kc == 0:
                            wq_tiles[nt + 1] = wq_pool.tile(
                                [P, ko_n, n_tile], MM_DT, tag="wq",
                                name=f"wq{nt + 1}")
                        load_w_krange(wq_tiles[nt + 1], nt + 1,
                                      kc * w_kchunk, w_kchunk, kc)
                    last_kc = kc == w_slices - 1
                    for mt in range(m_tiles):
                        for ko in range(kc * w_kchunk, (kc + 1) * w_kchunk):
                            nc.tensor.matmul(
                                ps_tiles[mt][:],
                                xt16[:, ko, ds(mt * P, P)],
                                wq[:, ko, :],
                                start=(ko == 0),
                                stop=(ko == ko_n - 1),
                            )
                        if last_kc:
                            yt = out_pool.tile([P, n_tile], mybir.dt.float32,
                                               tag="yt")
                            nc.vector.tensor_add(
                                yt[:], ps_tiles[mt][:],
                                bias_sb[:, ds(nt * n_tile, n_tile)])
                            nc.sync.dma_start(
                                y3[:, mt, ds(nt * n_tile, n_tile)], yt[:])

    nc.compile()
    return nc


_NC_CACHE = {}


def _get_nc():
    if "nc" not in _NC_CACHE:
        _NC_CACHE["nc"] = build_nc()
    return _NC_CACHE["nc"]


M_LOC = B_FULL // R
N_LOC = N_FULL // C
N_TILE = 512


def wire_x(x_shard, k=K_FULL):
    """[m, k] f32 -> partition-major [P, ko, m] fp16."""
    m = x_shard.shape[0]
    return np.ascontiguousarray(
        x_shard.reshape(m, k // P, P).transpose(2, 1, 0)).astype(np.float16)


def wire_w(w_shard, k=K_FULL, n_tile=N_TILE):
    """[k, n] f32 -> partition-major [P, nt, ko, n_tile] bf16."""
    n = w_shard.shape[1]
    return np.ascontiguousarray(
        w_shard.reshape(k // P, P, n // n_tile, n_tile).transpose(1, 2, 0, 3)
    ).astype(ml_dtypes.bfloat16)


def wire_b(b_shard):
    """[n] f32 -> broadcast [P, n] f32."""
    return np.ascontiguousarray(
        np.broadcast_to(b_shard, (P, b_shard.shape[0])).astype(np.float32))


def make_in_maps(x, W, b):
    """Host-side shard + layout prep: per-core input dicts."""
    x = np.ascontiguousarray(np.asarray(x, dtype=np.float32))
    W = np.ascontiguousarray(np.asarray(W, dtype=np.float32))
    b = np.ascontiguousarray(np.asarray(b, dtype=np.float32))
    in_maps = []
    for core in range(N_CORES):
        i, j = divmod(core, C)
        in_maps.append({
            "xT": wire_x(x[i * M_LOC:(i + 1) * M_LOC, :]),
            "w": wire_w(W[:, j * N_LOC:(j + 1) * N_LOC]),
            "bias": wire_b(b[j * N_LOC:(j + 1) * N_LOC]),
        })
    return in_maps


def gather_out(results):
    """Assemble per-core y shards into the full [4096, 4096] output."""
    y = np.empty((B_FULL, N_FULL), np.float32)
    for core in range(N_CORES):
        i, j = divmod(core, C)
        y[i * M_LOC:(i + 1) * M_LOC, j * N_LOC:(j + 1) * N_LOC] = (
            results[core]["y"])
    return y


def kernel(x, W, b):
    nc = _get_nc()
    in_maps = make_in_maps(x, W, b)
    res = bass_utils.run_bass_kernel_spmd(nc, in_maps, core_ids=list(range(N_CORES)))
    return gather_out(res.results)
